# revision 38
# baseline (speedup 1.0000x reference)
"""Cross-attention Trainium2 kernel, SPMD over 8 NeuronCores.

Sharding: core c = b*4 + g handles batch b (of 2) and head-group g (of 4,
4 heads each) — data parallel on B, Megatron tensor parallel on heads:
W_qkv column-sliced, W_proj row-sliced, partial outputs summed on host.

Per-core dataflow (all matmuls bf16 operands, fp32 PSUM accumulation):
  - host pre-transposes x/context to [D, L] and pre-casts weights to bf16
  - K^T[n,key] = Wk.T @ c^T ; V[key,n] = c @ Wv (+bias via K=1 matmul)
  - Q^T[n,tok] = Wq.T @ x^T
  - S^T[key,tok] = K_h @ Q_h^T ; P^T = exp(scale*S^T) on ACT
  - O[tok,hd]+denom = P @ [V|1], normalize rows on DVE
  - O^T via PE transpose; out[tok,dout] = O @ Wp

Scheduling: the exp of the 2048x2048x4 score tiles costs ~612ns per
[128,512] tile on ACT (~157us/core total) while the S matmul feeding it
costs only 213ns on PE — ACT is the attention-phase bottleneck.  So the
whole kernel is software-pipelined as one stream: S-matmul+exp steps form
a pacing skeleton and all other PE work (Q projections of later chunks,
PV, transposes, out-projection of earlier chunks) is emitted from a filler
queue between steps via a debt counter, keeping PE busy while ACT drains.
K/V projections run first (exp-free) while DMA streams context chunks.
"""

import numpy as np
import ml_dtypes

import concourse.bass as bass
import concourse.bacc as bacc
import concourse.mybir as mybir
from concourse.bass import ts
from concourse.masks import make_identity
from concourse.tile import TileContext

DIM = 2048
NUM_HEADS = 16
HEAD_DIM = 128
B, L = 2, 2048
GPB = 4                  # head-groups per batch (cores per batch)
HPC = NUM_HEADS // GPB   # heads per core = 4
NPC = HPC * HEAD_DIM     # per-core projection width = 512
N_CORES = 8

F32 = mybir.dt.float32
BF16 = mybir.dt.bfloat16
BF16_NP = ml_dtypes.bfloat16


def build_bass(dim=DIM, seq=L, hpc=HPC, hd=HEAD_DIM, repeat=1):
    """Build the per-core SPMD Bass program (parameterized for testing)."""
    npc = hpc * hd
    KT = dim // 128      # contraction tiles over model dim
    LT = seq // 128      # key 128-tiles
    TCH = seq // 512     # token 512-chunks
    scale = float(hd) ** -0.5
    Exp = mybir.ActivationFunctionType.Exp

    nc = bacc.Bacc()
    xT = nc.dram_tensor("xT", [dim, seq], BF16, kind="ExternalInput")
    cT = nc.dram_tensor("cT", [dim, seq], BF16, kind="ExternalInput")
    wq = nc.dram_tensor("wq", [dim, npc], BF16, kind="ExternalInput")
    wk = nc.dram_tensor("wk", [dim, npc], BF16, kind="ExternalInput")
    wv = nc.dram_tensor("wv", [dim, npc], BF16, kind="ExternalInput")
    bq = nc.dram_tensor("bq", [128, hpc], F32, kind="ExternalInput")
    wp = nc.dram_tensor("wp", [npc, dim], BF16, kind="ExternalInput")
    out = nc.dram_tensor("out", [seq, dim], BF16, kind="ExternalOutput")

    with TileContext(nc) as tc:
        with (
            tc.tile_pool(name="psS", bufs=2, space="PSUM") as psS,
            tc.tile_pool(name="psmm", bufs=2, space="PSUM") as psmm,
            tc.tile_pool(name="psov", bufs=2, space="PSUM") as psov,
            tc.tile_pool(name="small", bufs=4) as small,
            tc.tile_pool(name="obp", bufs=3) as obp,
        ):
            for _rep in range(repeat):
                _build_body(
                    nc, tc, psS, psmm, psov, small, obp,
                    xT, cT, wq, wk, wv, bq, wp, out,
                    dim, seq, hpc, hd, npc, KT, LT, TCH, scale, Exp,
                )

    nc.compile()
    return nc


def _build_body(
    nc, tc, psS, psmm, psov, small, obp,
    xT, cT, wq, wk, wv, bq, wp, out,
    dim, seq, hpc, hd, npc, KT, LT, TCH, scale, Exp,
):
    with (
        tc.tile_pool(name="res", bufs=1) as res,
        tc.tile_pool(name="stream", bufs=2) as stream,
        tc.tile_pool(name="ptp", bufs=3) as ptp,
        tc.tile_pool(name="qtp", bufs=3) as qtp,
        tc.tile_pool(name="opool", bufs=2) as opool,
        tc.tile_pool(name="otp", bufs=2) as otp,
    ):
        KTl = res.tile([128, hpc, seq], BF16)    # [hd, h, key]
        V = res.tile([128, LT, hpc, 129], BF16)  # [key, ktile, h, hd+1(ones)+pad]
        Wq_sb = res.tile([128, KT, npc], BF16)
        Wk_sb = res.tile([128, KT, npc], BF16)
        Wv_sb = res.tile([128, KT, npc], BF16)
        Wp_sb = res.tile([128, hpc, dim], BF16)
        bq_sb = res.tile([128, hpc], F32)
        ident = res.tile([128, 128], BF16)
        warm = res.tile([1, 2], F32)

        nc.vector.memset(V[:, :, :, 128:129], 1.0)
        nc.vector.memset(warm[:], 0.0)
        make_identity(nc, ident[:])
        # preload the Exp table so the 1.3us table load is off critical path
        nc.scalar.activation(warm[0:1, 0:1], warm[0:1, 1:2], Exp)

        wq_r = wq[:, :].rearrange("(kt p) n -> p kt n", p=128)
        wk_r = wk[:, :].rearrange("(kt p) n -> p kt n", p=128)
        wv_r = wv[:, :].rearrange("(kt p) n -> p kt n", p=128)

        # ---- DMA, critical-path order.  First matmul needs Wk tile0 +
        # ctx-chunk0 tile0, so those interleave per k-tile.
        cst = [None] * TCH
        xst = [None] * TCH
        cst[0] = stream.tile([128, KT, 512], BF16, tag="stream", name="cst0")
        c0_r = cT[:, 0:512].rearrange("(kt p) n -> p kt n", p=128)
        for g in range(0, KT, 2):
            nc.sync.dma_start(cst[0][:, g : g + 2, :], c0_r[:, g : g + 2, :])
            nc.sync.dma_start(Wk_sb[:, g : g + 2, :], wk_r[:, g : g + 2, :])
        nc.sync.dma_start(Wv_sb[:], wv_r[:, :, :])

        def load_chunk(src, t):
            tile = stream.tile([128, KT, 512], BF16, tag="stream", name=f"ch_{src is xT}_{t}")
            src_r = src[:, ts(t, 512)].rearrange("(kt p) n -> p kt n", p=128)
            half = KT // 2
            nc.sync.dma_start(tile[:, :half, :], src_r[:, :half, :])
            nc.sync.dma_start(tile[:, half:, :], src_r[:, half:, :])
            return tile

        cst[1] = load_chunk(cT, 1)
        nc.sync.dma_start(Wq_sb[:], wq_r[:, :, :])
        nc.sync.dma_start(bq_sb[:], bq[:, :])
        nc.sync.dma_start(
            Wp_sb[:], wp[:, :].rearrange("(h p) d -> p h d", p=128)
        )

        # ---- emit helpers ----
        def k_chain(c, h):
            ps = psmm.tile([128, 512], F32, tag="mm512")
            for kt in range(KT):
                nc.tensor.matmul(
                    ps[:], Wk_sb[:, kt, ts(h, 128)], cst[c][:, kt, :],
                    start=(kt == 0), stop=(kt == KT - 1),
                )
            nc.vector.tensor_copy(KTl[:, h, ts(c, 512)], ps[:])

        def v_chain(c, j):
            kt2 = c * 4 + j
            ps = psmm.tile([128, 512], F32, tag="mm512")
            # b_v is folded into the host-side output bias (softmax rows
            # sum to 1, so V's bias contributes exactly b_v @ W_proj)
            for kt in range(KT):
                nc.tensor.matmul(
                    ps[:, :npc], cst[c][:, kt, ts(j, 128)], Wv_sb[:, kt, :],
                    start=(kt == 0), stop=(kt == KT - 1),
                )
            nc.vector.tensor_copy(
                V[:, kt2, :, 0:128],
                ps[:, :npc].rearrange("p (h c) -> p h c", h=hpc),
            )

        qts = {}

        def q_item(t, h):
            def f():
                if t not in qts:
                    qts[t] = qtp.tile([128, hpc, 512], BF16, tag="qt", name=f"qt_{t}")
                ps = psmm.tile([128, 512], F32, tag="mm512")
                for kt in range(KT):
                    nc.tensor.matmul(
                        ps[:], Wq_sb[:, kt, ts(h, 128)], xst[t][:, kt, :],
                        start=(kt == 0), stop=(kt == KT - 1),
                    )
                nc.vector.tensor_scalar_add(qts[t][:, h, :], ps[:], bq_sb[:, h : h + 1])
            return (3400.0, f)

        o_cur = {}
        ot_cur = {}
        pts = {}

        def pv_item(t, h, j, flush=False):
            def f():
                if t not in o_cur:
                    o_cur[t] = opool.tile([128, 4, 512], BF16, tag="o", name=f"o_{t}")
                PT = pts[(t, h)]
                po = psov.tile([128, 130], F32, tag="po")
                for kt2 in range(LT):
                    nc.tensor.matmul(
                        po[:, 0:129],
                        PT[:, kt2, ts(j, 128)],
                        V[:, kt2, h, 0:129],
                        start=(kt2 == 0), stop=(kt2 == LT - 1),
                    )
                rc = small.tile([128, 1], F32, tag="recip")
                nc.vector.reciprocal(rc[:], po[:, 128:129])
                nc.vector.tensor_scalar_mul(
                    o_cur[t][:, j, ts(h, 128)], po[:, 0:128], rc[:]
                )
            return (870.0, f)

        def transp_item(t, j, flush=False):
            def f():
                if t not in ot_cur:
                    ot_cur[t] = otp.tile([128, hpc, 512], BF16, tag="ot", name=f"ot_{t}")
                ps = psmm.tile([128, 512], BF16, tag="mm512")
                for h in range(hpc):
                    nc.tensor.transpose(
                        ps[:, ts(h, 128)], o_cur[t][:, j, ts(h, 128)], ident[:]
                    )
                nc.vector.tensor_copy(
                    ot_cur[t][:, :, ts(j, 128)],
                    ps[:, :].rearrange("p (h c) -> p h c", h=hpc),
                )
            return (220.0, f)

        def outproj_item(t, j, dc, flush=False):
            def f():
                tt = t * 4 + j
                ps = psmm.tile([128, 512], F32, tag="mm512")
                for h in range(hpc):
                    nc.tensor.matmul(
                        ps[:],
                        ot_cur[t][:, h, ts(j, 128)],
                        Wp_sb[:, h, ts(dc, 512)],
                        start=(h == 0), stop=(h == hpc - 1),
                    )
                ob = obp.tile([128, 512], BF16, tag="ob")
                nc.vector.tensor_copy(ob[:], ps[:])
                nc.sync.dma_start(out[ts(tt, 128), ts(dc, 512)], ob[:])
            return (860.0, f)

        # ---- phase A: K/V projections (exp-free), streaming ctx chunks ----
        for c in range(TCH):
            for h in range(hpc):
                k_chain(c, h)
            for j in range(hpc):
                v_chain(c, j)
            if c + 2 < TCH:
                cst[c + 2] = load_chunk(cT, c + 2)
            else:
                xst[c + 2 - TCH] = load_chunk(xT, c + 2 - TCH)

        # Q chunk 0 directly (xst[0] prefetched during K/V of chunk 3)
        for h in range(hpc):
            q_item(0, h)[1]()
        xst[2] = load_chunk(xT, 2)

        # ---- phase B: paced units over token chunks ----
        queue = []      # list of (est_pe_ns, emit_fn, key)
        debt = [0.0]

        def pay():
            while queue and debt[0] > 0:
                cost, fn, _ = queue.pop(0)
                fn()
                debt[0] -= cost

        def force(key):
            # emit all queued items with this key, preserving order of rest
            rest = []
            for ent in queue:
                if ent[2] == key:
                    ent[1]()
                else:
                    rest.append(ent)
            queue[:] = rest

        def push_tail(t, flush=False):
            # transp(j) ahead of outproj(j) with lag 1 so PE outproj work
            # covers the DVE copy latency of the next transpose
            items = []
            for j in range(4):
                items.append(transp_item(t, j, flush))
            seqd = [items[0], items[1]]
            for j in range(4):
                for dc in range(dim // 512):
                    seqd.append(outproj_item(t, j, dc, flush))
                if j + 2 < 4:
                    seqd.append(items[j + 2])
            for c, f in seqd:
                queue.append((c, f, ("tail", t)))

        def unit(t):
            # external filler: Q of chunk t+1, tail (transpose+outproj) of t-1
            if t == 1:
                xst[3] = load_chunk(xT, 3)
            if t + 1 < TCH:
                for h in range(hpc):
                    c, f = q_item(t + 1, h)
                    queue.append((c, f, ("q", t + 1)))
            if t - 1 >= 0:
                push_tail(t - 1)
            force(("q", t))
            # skeleton: S matmul + exp steps, with forced PV drains so the
            # 3-deep PT ring never blocks ACT on an un-emitted PV (deadlock)
            for h in range(hpc):
                if 0 <= h <= 2 and t - 1 >= 0:
                    force(("pv", t - 1, h + 1))
                if h == 3:
                    force(("pv", t, 0))
                if h >= 2:
                    for j in range(3, -1, -1):
                        c, f = pv_item(t, h - 2, j)
                        queue.insert(0, (c, f, ("pv", t, h - 2)))
                pts[(t, h)] = ptp.tile([128, LT, 512], BF16, tag="pt", name=f"pt_{t}_{h}")
                PTr = pts[(t, h)][:, :, :].rearrange("p kt n -> p (kt n)")
                for kt2 in range(0, LT, 2):
                    ps = psS.tile([128, 1024], F32, tag="s")
                    for u in range(2):
                        nc.tensor.matmul(
                            ps[:, ts(u, 512)],
                            KTl[:, h, ts(kt2 + u, 128)],
                            qts[t][:, h, :],
                            start=True, stop=True,
                        )
                    nc.scalar.activation(
                        PTr[:, ts(kt2 // 2, 1024)], ps[:], Exp, scale=scale
                    )
                    debt[0] += (1095.0 - 426.0) * 1.6
                    pay()
            # PV of the last two heads rides the queue into the next unit
            for h in (3, 2):
                for j in range(3, -1, -1):
                    c, f = pv_item(t, h, j, flush=(t == TCH - 1))
                    queue.insert(0, (c, f, ("pv", t, h)))

        for t in range(TCH):
            unit(t)
        # drain: tail of the last chunk + whatever is still queued
        push_tail(TCH - 1, flush=True)
        while queue:
            _, fn, _ = queue.pop(0)
            fn()


def make_in_maps(x, context, W_qkv, b_qkv, W_proj):
    """Shard + pre-layout full inputs into per-core input maps."""
    x = np.asarray(x, dtype=np.float32)
    context = np.asarray(context, dtype=np.float32)
    W_qkv = np.asarray(W_qkv, dtype=np.float32)
    b_qkv = np.asarray(b_qkv, dtype=np.float32)
    W_proj = np.asarray(W_proj, dtype=np.float32)

    in_maps = []
    for c in range(N_CORES):
        b, g = divmod(c, GPB)
        n0 = g * NPC
        xTb = np.ascontiguousarray(x[b].T).astype(BF16_NP)
        cTb = np.ascontiguousarray(context[b].T).astype(BF16_NP)
        in_maps.append(
            {
                "xT": xTb,
                "cT": cTb,
                "wq": np.ascontiguousarray(W_qkv[:, n0 : n0 + NPC]).astype(BF16_NP),
                "wk": np.ascontiguousarray(
                    W_qkv[:, DIM + n0 : DIM + n0 + NPC]
                ).astype(BF16_NP),
                "wv": np.ascontiguousarray(
                    W_qkv[:, 2 * DIM + n0 : 2 * DIM + n0 + NPC]
                ).astype(BF16_NP),
                "bq": np.ascontiguousarray(
                    b_qkv[n0 : n0 + NPC].reshape(HPC, 128).T
                ).astype(np.float32),
                "wp": np.ascontiguousarray(W_proj[n0 : n0 + NPC, :]).astype(BF16_NP),
            }
        )
    return in_maps


_NC_CACHE = {}


def kernel(x, context, W_qkv, b_qkv, W_proj, b_proj, _trace=False):
    from concourse.bass_utils import run_bass_kernel_spmd

    b_proj = np.asarray(b_proj, dtype=np.float32)
    in_maps = make_in_maps(x, context, W_qkv, b_qkv, W_proj)

    if "nc" not in _NC_CACHE:
        _NC_CACHE["nc"] = build_bass()
    nc = _NC_CACHE["nc"]

    res = run_bass_kernel_spmd(nc, in_maps, list(range(N_CORES)), trace=_trace)
    results = res.results

    out = np.zeros((B, L, DIM), dtype=np.float32)
    for c in range(N_CORES):
        b = c // GPB
        out[b] += results[c]["out"].astype(np.float32)
    # b_k cancels in softmax; b_v contributes b_v @ W_proj (softmax rows
    # sum to 1) — both folded here instead of inside the kernel
    W_proj = np.asarray(W_proj, dtype=np.float32)
    b_qkv = np.asarray(b_qkv, dtype=np.float32)
    out += (b_proj + b_qkv[2 * DIM :] @ W_proj)[None, None, :]
    if _trace:
        return out, res
    return out
